# revision 33
# baseline (speedup 1.0000x reference)
"""nn_DirAttention kernel for 8 Trainium2 NeuronCores.

Strategy: data-parallel over batch (B=8, one batch element per core).
Per core, the directional attention

    ah[o,i,j] = sum_k Wc[o,k] * Qh[k,i] * Kh[k,j]   (k = C*L = 4096)

is computed by materialising G[k,(j,i)] = Kh[k,j]*Qh[k,i] per 128-row
k-block on the Vector engine (outer-product broadcast via a
column-duplicated K so every operand presents dense bf16 pairs to the
DVE -> 2x mode), then accumulating ah = Wc' @ G on the PE with even/odd
k-blocks on the two halves of the array.  Softmax over the channel
(partition) axis uses an ACT exp with per-partition bias bc, a
ones-matmul for the column sums, a 64-lane reciprocal via a DRAM
shuffle, and a DMA partition-broadcast of 1/Z.  The 3x3 conv runs as
shifted accumulating matmuls over zero-padded SBUF images, with both
image halves accumulating into one PSUM pass per output chunk.
BatchNorm is folded into the conv weights on the host.

The projections read a single x tile with parity-offset access
patterns (no materialised shifted copies), the Z sums contract K=64,
the conv's second image pass is K=64 (no zero rows), the BN shift
runs on the ACT engine, and y is written back in bf16.
"""

import sys

for _p in ("/opt/trn_rl_repo",):
    if _p not in sys.path:
        sys.path.append(_p)

import numpy as np
import ml_dtypes

import concourse.bacc as bacc
import concourse.bass as bass
import concourse.mybir as mybir
import concourse.tile as tile
from concourse.bass_utils import run_bass_kernel_spmd

BF16 = mybir.dt.bfloat16
F32 = mybir.dt.float32
B, C, L = 8, 64, 64
N = L * L  # 4096
NKB = 32  # 128-row k-blocks in the C*L contraction
BN_EPS = 1e-5
PAD = L + 2  # 66, padded row stride for the conv images

_CACHE = {}


def _build_nc(debug=False):
    nc = bacc.Bacc(target_bir_lowering=False)

    # ---- DRAM parameters -------------------------------------------------
    xbf = nc.dram_tensor("xbf", [C, N], BF16, kind="ExternalInput")
    x2bf = nc.dram_tensor("x2bf", [128, N // 2], BF16, kind="ExternalInput")
    wqt_d = nc.dram_tensor("wqt", [128, 64], BF16, kind="ExternalInput")
    wkt_d = nc.dram_tensor("wkt", [128, 64], BF16, kind="ExternalInput")
    wcpt = nc.dram_tensor("wcpt", [128, NKB, 64], BF16, kind="ExternalInput")
    woa = nc.dram_tensor("woa", [128, 9, 64], BF16, kind="ExternalInput")
    wob = nc.dram_tensor("wob", [64, 9, 64], BF16, kind="ExternalInput")
    bqq_d = nc.dram_tensor("bqq", [128, 1], F32, kind="ExternalInput")
    bkk_d = nc.dram_tensor("bkk", [128, 1], F32, kind="ExternalInput")
    bc_d = nc.dram_tensor("bc", [64, 1], F32, kind="ExternalInput")
    bo_d = nc.dram_tensor("bo_eff", [64, 1], F32, kind="ExternalInput")
    dv_d = nc.dram_tensor("d_vec", [64, 1], F32, kind="ExternalInput")
    ident_d = nc.dram_tensor("ident", [128, 64], BF16, kind="ExternalInput")
    y = nc.dram_tensor("y", [C, N], BF16, kind="ExternalOutput")
    taps = {}
    if debug:
        for nm, shp, dt in [
            ("t_att", [64, N], F32), ("t_z", [64, N], F32),
            ("t_hatt", [64, N], F32), ("t_watt", [64, N], F32),
        ]:
            taps[nm] = nc.dram_tensor(nm, shp, dt, kind="ExternalOutput")

    from contextlib import ExitStack
    with tile.TileContext(nc) as tc, ExitStack() as _es:
        consts = _es.enter_context(tc.tile_pool(name="consts", bufs=1))
        qk = _es.enter_context(tc.tile_pool(name="qk", bufs=1))
        work = _es.enter_context(tc.tile_pool(name="work", bufs=2))
        gpool = _es.enter_context(tc.tile_pool(name="gpool", bufs=12))
        dpool = _es.enter_context(tc.tile_pool(name="dscratch", bufs=2, space="DRAM"))

        # ---- constant loads ---------------------------------------------
        xs = consts.tile([64, N], BF16)     # x image for apply/conv stages
        x2 = consts.tile([128, N // 2], BF16)  # x col-halves stacked: fast load
        wq_sb = consts.tile([128, 64], BF16)
        wk_sb = consts.tile([128, 64], BF16)
        wc_sb = consts.tile([128, NKB, 64], BF16)
        woa_sb = consts.tile([128, 9, 64], BF16)
        wob_sb = consts.tile([64, 9, 64], BF16)
        bqq = consts.tile([128, 1], F32)
        bkk = consts.tile([128, 1], F32)
        bcv = consts.tile([64, 1], F32)
        bov2 = consts.tile([128, 1], F32)
        dvv2 = consts.tile([128, 1], F32)
        ones = consts.tile([128, 1], BF16)
        ident_sb = consts.tile([128, 64], BF16)
        warm = consts.tile([128, 512], BF16)

        # DMA rate is ~per-partition-bytes, so the proj-gating x2 image is
        # split by columns across two queues; wc arrives in k-block chunks
        # (the attention matmuls consume them in order); the [64, 4096] xs
        # copy (apply/conv stages, needed much later) trickles in behind.
        nc.sync.dma_start(out=x2[:, 0:1024], in_=x2bf[:, 0:1024])
        nc.gpsimd.dma_start(out=x2[:, 1024:2048], in_=x2bf[:, 1024:2048])
        nc.scalar.dma_start(out=wq_sb[:], in_=wqt_d[:])
        nc.scalar.dma_start(out=wk_sb[:], in_=wkt_d[:])
        nc.scalar.dma_start(out=bqq[:], in_=bqq_d[:])
        nc.scalar.dma_start(out=bkk[:], in_=bkk_d[:])
        nc.scalar.dma_start(out=bcv[:], in_=bc_d[:])
        nc.sync.dma_start(out=wc_sb[:, 0:8, :], in_=wcpt[:, 0:8, :])
        for wch in range(1, 4):
            nc.gpsimd.dma_start(out=wc_sb[:, 8 * wch:8 * (wch + 1), :],
                                in_=wcpt[:, 8 * wch:8 * (wch + 1), :])
        nc.scalar.dma_start(out=xs[:, 2048:4096], in_=xbf[:, 2048:4096])
        nc.sync.dma_start(out=xs[:, 0:2048], in_=xbf[:, 0:2048])
        nc.gpsimd.dma_start(out=woa_sb[:], in_=woa[:])
        nc.gpsimd.dma_start(out=wob_sb[:], in_=wob[:])
        nc.gpsimd.dma_start(out=bov2[0:64], in_=bo_d[:])
        nc.gpsimd.dma_start(out=bov2[64:128], in_=bo_d[:])
        nc.gpsimd.dma_start(out=dvv2[0:64], in_=dv_d[:])
        nc.gpsimd.dma_start(out=dvv2[64:128], in_=dv_d[:])
        nc.gpsimd.dma_start(out=ident_sb[:], in_=ident_d[:])
        nc.vector.memset(ones[:], 1.0)
        nc.vector.memset(warm[:], 1.0)

        # conv image buffers (zero ring borders; interiors fully written)
        catA = consts.tile([128, PAD * PAD], BF16)  # rows 0-63 x, 64-127 w_att
        catB = consts.tile([64, PAD * PAD], BF16)   # h_att
        for t, p in ((catA, 128), (catB, 64)):
            nc.vector.memset(t[:, 0:PAD], 0.0)
            nc.vector.memset(t[:, (PAD - 1) * PAD:PAD * PAD], 0.0)
            nc.vector.memset(
                bass.AP(tensor=t.tensor, offset=t.offset + PAD,
                        ap=[t.ap[0], [PAD, L]]), 0.0)
            nc.vector.memset(
                bass.AP(tensor=t.tensor, offset=t.offset + PAD + L + 1,
                        ap=[t.ap[0], [PAD, L]]), 0.0)

        def pad_interior_ap(t, p0, p1, row0=0, nrows=L):
            base = t[p0:p1, :]
            return bass.AP(tensor=base.tensor,
                           offset=base.offset + (row0 + 1) * PAD + 1,
                           ap=[base.ap[0], [PAD, nrows], [1, L]])

        # x part of the conv image
        nc.sync.dma_start(out=pad_interior_ap(catA, 0, 64), in_=xs[:])

        # ---- projections -------------------------------------------------
        # Per direction: Q [128, 32, 64] (block kb = spatial pair, partition
        # = (parity, channel)), Kdup [128, 32, 64, 2] (K duplicated pairs).
        # The parity halves read the single x tile at +1 / +64 offsets.
        q_t = {d: qk.tile([128, NKB, 64], BF16, tag=f"q{d}", name=f"q_{d}") for d in "hw"}
        kd_t = {d: qk.tile([128, NKB, 64, 2], BF16, tag=f"k{d}", name=f"kd_{d}") for d in "hw"}

        with tc.tile_pool(name="projps", bufs=6, space="PSUM") as pps, \
             tc.tile_pool(name="warmps", bufs=1, space="PSUM") as wps:
            # clock-warming dummy matmuls (PE otherwise idles until x lands)
            wt = wps.tile([1, 512], F32, tag="warm")
            for _ in range(8):
                nc.tensor.matmul(out=wt[:], lhsT=warm[:, 0:1], rhs=warm[:],
                                 start=True, stop=True)
            # w first: its projections read the fast-loading stacked x2
            # image ((h<32, h>=32) on partition halves, clean t8-level
            # split).  h's projections read the slowly-arriving xs and run
            # during w's G phase, well off the critical path.
            for d in "wh":
                for t8 in range(4):  # 8 g-blocks per psum tile
                    for proj in "qk":
                        wsb = wq_sb if proj == "q" else wk_sb
                        bias = bqq if proj == "q" else bkk
                        ps = pps.tile([128, 8, 64], F32, tag="proj")
                        for half in range(2):  # 4 g per matmul
                            g0 = t8 * 8 + half * 4
                            for par in range(2):
                                if d == "h":
                                    rhs = bass.AP(
                                        tensor=xs.tensor,
                                        offset=xs.offset + 2 * g0 + par,
                                        ap=[xs.ap[0], [2, 4], [64, 64]])
                                    nc.tensor.matmul(
                                        out=ps[par * 64:(par + 1) * 64,
                                               half * 4:(half + 1) * 4, :],
                                        lhsT=wsb[0:64, :], rhs=rhs,
                                        start=True, stop=True,
                                        skip_group_check=True,
                                        tile_position=(0, par * 64))
                                    continue
                                hi = t8 >= 2
                                p0 = 64 * hi
                                xb = x2[p0:p0 + 64, :]
                                rhs = bass.AP(
                                    tensor=xb.tensor,
                                    offset=xb.offset + 128 * g0 + 64 * par
                                    - hi * 2048,
                                    ap=[xb.ap[0], [128, 4], [1, 64]])
                                nc.tensor.matmul(
                                    out=ps[par * 64:(par + 1) * 64,
                                           half * 4:(half + 1) * 4, :],
                                    lhsT=wsb[p0:p0 + 64, :], rhs=rhs,
                                    start=True, stop=True,
                                    skip_group_check=True,
                                    tile_position=(p0, par * 64))
                        if proj == "q":
                            nc.scalar.activation(
                                out=q_t[d][:, t8 * 8:(t8 + 1) * 8, :], in_=ps[:],
                                func=mybir.ActivationFunctionType.Identity,
                                bias=bias[:], scale=1.0)
                        else:
                            for dup in range(2):
                                dst = bass.AP(
                                    tensor=kd_t[d].tensor,
                                    offset=kd_t[d].offset + t8 * 8 * 128 + dup,
                                    ap=[kd_t[d].ap[0], [128, 8], [2, 64]])
                                nc.scalar.activation(
                                    out=dst, in_=ps[:],
                                    func=mybir.ActivationFunctionType.Identity,
                                    bias=bias[:], scale=1.0)

        # ---- attention + softmax + apply + conv --------------------------
        att_t = {d: work.tile([64, L, L], BF16, tag=f"att{d}", bufs=1,
                              name=f"att_{d}") for d in "hw"}
        hat_t = {"w": work.tile([64, N], BF16, tag="hatw", bufs=1, name="hat_w")}

        cv_tiles = {}

        def conv_A(cps, rps):
            # catA half (x + h_att): runs during the w Z-chain latency
            for rp in rps:
                cv_tiles[rp] = cv = cps.tile([128, 512], F32, tag="cv", name="cv")
                for tap in range(9):
                    dy, dx = tap // 3, tap % 3
                    for half in range(2):
                        r = rp * 2 + half
                        off = (r * 8 + dy) * PAD + dx
                        rhs = bass.AP(tensor=catA.tensor, offset=catA.offset + off,
                                      ap=[catA.ap[0], [PAD, 8], [1, 64]])
                        nc.tensor.matmul(out=cv[half * 64:(half + 1) * 64, :],
                                         lhsT=woa_sb[:, tap, :], rhs=rhs,
                                         start=(tap == 0), stop=False,
                                         skip_group_check=True,
                                         tile_position=(0, half * 64))

        def conv_tail(cps, rps):
            # catB half (w_att, K=64) accumulates onto the A-pass result
            for rp in rps:
                cv = cv_tiles[rp]
                for tap in range(9):
                    dy, dx = tap // 3, tap % 3
                    for half in range(2):
                        r = rp * 2 + half
                        off = (r * 8 + dy) * PAD + dx
                        rhs = bass.AP(tensor=catB.tensor, offset=catB.offset + off,
                                      ap=[catB.ap[0], [PAD, 8], [1, 64]])
                        nc.tensor.matmul(out=cv[half * 64:(half + 1) * 64, :],
                                         lhsT=wob_sb[:, tap, :], rhs=rhs,
                                         start=False, stop=(tap == 8),
                                         skip_group_check=True,
                                         tile_position=(0, half * 64))
                ysb = work.tile([128, 512], F32, tag="ysb", name="ysb")
                nc.scalar.activation(out=ysb[0:64, :], in_=cv[0:64, :],
                                     func=mybir.ActivationFunctionType.Relu,
                                     bias=bov2[0:64], scale=1.0)
                nc.scalar.activation(out=ysb[64:128, :], in_=cv[64:128, :],
                                     func=mybir.ActivationFunctionType.Relu,
                                     bias=bov2[64:128], scale=1.0)
                ysb2 = work.tile([128, 512], BF16, tag="ysb2", name="ysb2")
                nc.scalar.activation(out=ysb2[:], in_=ysb[:],
                                     func=mybir.ActivationFunctionType.Identity,
                                     bias=dvv2[:], scale=1.0)
                nc.sync.dma_start(out=y[:, (2 * rp) * 512:(2 * rp + 1) * 512],
                                  in_=ysb2[0:64, :])
                nc.sync.dma_start(out=y[:, (2 * rp + 1) * 512:(2 * rp + 2) * 512],
                                  in_=ysb2[64:128, :])

        with tc.tile_pool(name="ahps", bufs=1, space="PSUM") as aps, \
             tc.tile_pool(name="cvps", bufs=4, space="PSUM") as cps:

            def g_chain(d, jh, interleave=()):
                # G production + ah accumulation for one (direction, column
                # half).  `interleave` maps kbp -> [fn] emitting deferred DVE
                # ops (reciprocals / apply muls of the PREVIOUS chain) into
                # the middle of this chain's G stream, so their input DMAs
                # have landed by the time the in-order DVE queue reaches
                # them.
                last = (d == "h" and jh == 1)
                q, kd = q_t[d], kd_t[d]
                ah = aps.tile([128, 2048], F32, tag="ah", name="ah")
                inter = dict(interleave)
                for kbp in range(NKB // 2):
                    for fn in inter.get(kbp, ()):
                        fn()
                    grhs = {}
                    for half in range(2):
                        kb = kbp * 2 + half
                        g = gpool.tile([128, 32, 64], BF16, tag="g", name=f"g{half}")
                        # G[k, j, i] = K[k,j] * Q[k,i] (2x-mode paired APs)
                        in0 = bass.AP(
                            tensor=kd.tensor,
                            offset=kd.offset + kb * 128 + jh * 64,
                            ap=[kd.ap[0], [2, 32], [0, 32], [1, 2]])
                        in1 = bass.AP(
                            tensor=q.tensor, offset=q.offset + kb * 64,
                            ap=[q.ap[0], [0, 32], [2, 32], [1, 2]])
                        gout = bass.AP(
                            tensor=g.tensor, offset=g.offset,
                            ap=[g.ap[0], [64, 32], [2, 32], [1, 2]])
                        nc.vector.tensor_mul(out=gout, in0=in0, in1=in1)
                        grhs[half] = g[:].rearrange("p a b -> p (a b)")
                    for ns in range(4):
                        for half in range(2):
                            kb = kbp * 2 + half
                            opart = 0 if last else half * 64
                            nc.tensor.matmul(
                                out=ah[opart:opart + 64, ns * 512:(ns + 1) * 512],
                                lhsT=wc_sb[:, kb, :],
                                rhs=grhs[half][:, ns * 512:(ns + 1) * 512],
                                start=(kbp == 0 and (half == 0 or not last)),
                                stop=(kbp == NKB // 2 - 1 and ns == 3),
                                skip_group_check=True,
                                tile_position=(0, opart))
                if not last:
                    # fold the odd-half partial into the even-half region
                    # via an identity matmul (ACT copy, same partitions).
                    fold = work.tile([128, 2048], BF16, tag="fold", name="fold", bufs=2)
                    nc.scalar.copy(out=fold[64:128, :], in_=ah[64:128, :])
                    for ns in range(4):
                        nc.tensor.matmul(
                            out=ah[0:64, ns * 512:(ns + 1) * 512],
                            lhsT=ident_sb[64:128, :],
                            rhs=fold[64:128, ns * 512:(ns + 1) * 512],
                            start=False, stop=True,
                            skip_group_check=True,
                            tile_position=(64, 0))
                # exp with transposed read: ah[(j,i)] -> att[(i, j)]
                src = bass.AP(tensor=ah.tensor, offset=ah.offset,
                              ap=[[ah.ap[0][0], 64], [1, 64], [64, 32]])
                nc.scalar.activation(
                    out=att_t[d][:, :, jh * 32:(jh + 1) * 32], in_=src,
                    func=mybir.ActivationFunctionType.Exp,
                    bias=bcv[:], scale=1.0)

            def z_sums(att3, chunks, zs_dst):
                # Z column sums: K=64 ones-matmuls into four disjoint
                # column-groups (psum rows 0/32/64/96) of ONE bank-wide
                # tile, then a single strided DMA spreads them into the
                # [rows, 32/64-wide] zs block for the reciprocal.
                zt4 = cps.tile([128, 512], F32, tag="cv", name="zt4")
                for c4, (off, apf) in enumerate(chunks):
                    rhs = bass.AP(tensor=att3.tensor, offset=att3.offset + off,
                                  ap=[att3.ap[0]] + apf)
                    nc.tensor.matmul(out=zt4[32 * c4:32 * c4 + 1, :],
                                     lhsT=ones[0:64], rhs=rhs,
                                     start=True, stop=True,
                                     skip_group_check=True,
                                     tile_position=(0, 32 * c4))
                zsp = work.tile([128, 512], F32, tag="zsp", bufs=2, name="zsp")
                nc.scalar.copy(out=zsp[:], in_=zt4[:])
                nc.scalar.dma_start(
                    out=zs_dst,
                    in_=bass.AP(tensor=zsp.tensor, offset=zsp.offset,
                                ap=[[zsp.ap[0][0] * 32, 4], [1, 512]]))

            def z_mms_j(d, jh):
                # per-column-half Z sums, (i-major, 32 j) layout
                zs = work.tile([64, 32], F32, tag="zsj", bufs=2, name="zsj")
                z_sums(att_t[d],
                       [(c4 * 16 * 64 + jh * 32, [[64, 16], [1, 32]])
                        for c4 in range(4)], zs[:])
                return zs

            def rz_chain_j(zs):
                # reciprocal + DRAM-broadcast of 1/Z for one column half
                rzs = work.tile([64, 32], BF16, tag="rzsj", bufs=2, name="rzsj")
                with nc.allow_low_precision(reason="1/Z multiplier in bf16"):
                    nc.vector.reciprocal(out=rzs[:], in_=zs[:])
                rz = dpool.tile([64, 32], BF16, tag="rzdj")
                nc.scalar.dma_start(out=rz[:], in_=rzs[:])
                rzb = work.tile([64, 2048], BF16, tag="rzbj", bufs=2, name="rzbj")
                for qi, queue in enumerate((nc.sync, nc.scalar)):
                    queue.dma_start(
                        out=rzb[:, qi * 1024:(qi + 1) * 1024],
                        in_=bass.AP(tensor=rz.tensor, offset=rz.offset + qi * 1024,
                                    ap=[[0, 64], [32, 32], [1, 32]]))
                return rzb

            # ---- w direction: full-width softmax chain, deferred into the
            # h/jh0 G stream ----------------------------------------------
            g_chain("w", 0)
            g_chain("w", 1)
            att_w = att_t["w"][:].rearrange("p a b -> p (a b)")
            if debug:
                nc.sync.dma_start(out=taps["t_z"][:], in_=att_w[:])
            zs_w = work.tile([64, 64], F32, tag="zsw", bufs=1)
            for hb in range(2):
                z_sums(att_t["w"],
                       [((hb * 4 + c4) * 512, [[1, 512]]) for c4 in range(4)],
                       zs_w[hb * 32:(hb + 1) * 32, :])
            rzb_w = work.tile([64, N], BF16, tag="rzbw", bufs=1)
            tmp_w = work.tile([64, N], BF16, tag="tmpw", bufs=1)

            def w_recip():
                rzs = work.tile([64, 64], BF16, tag="rzsw", bufs=1)
                with nc.allow_low_precision(reason="1/Z multiplier in bf16"):
                    nc.vector.reciprocal(out=rzs[:], in_=zs_w[:])
                rz = dpool.tile([64, 64], BF16, tag="rzdw")
                nc.scalar.dma_start(out=rz[:], in_=rzs[:])
                for ch in range(2):
                    sl = slice(ch * 2048, (ch + 1) * 2048)
                    nc.sync.dma_start(
                        out=rzb_w[:, sl],
                        in_=bass.AP(tensor=rz.tensor, offset=rz.offset + ch * 2048,
                                    ap=[[0, 64], [64, 32], [1, 64]]))

            def w_tmp():
                nc.vector.tensor_mul(out=tmp_w[:], in0=att_w[:], in1=xs[:])

            def w_hat(ch):
                sl = slice(ch * 2048, (ch + 1) * 2048)
                nc.vector.tensor_mul(out=hat_t["w"][:, sl], in0=tmp_w[:, sl],
                                     in1=rzb_w[:, sl])
                nc.sync.dma_start(
                    out=pad_interior_ap(catA, 64, 128, row0=ch * 32, nrows=32),
                    in_=hat_t["w"][:, sl])

            # ---- h direction, column half 0 ------------------------------
            g_chain("h", 0, {1: [w_recip], 2: [w_tmp],
                             4: [lambda: w_hat(0)], 5: [lambda: w_hat(1)]})
            zs_h0 = z_mms_j("h", 0)
            conv_A(cps, [0, 1, 2])
            rzb_h0 = [None]
            tmp_h0 = work.tile([64, 2048], BF16, tag="tmph0", bufs=1)

            def h0_recip():
                rzb_h0[0] = rz_chain_j(zs_h0)

            def h0_tmp():
                in0 = bass.AP(tensor=att_t["h"].tensor, offset=att_t["h"].offset,
                              ap=[att_t["h"].ap[0], [64, 64], [1, 32]])
                in1 = bass.AP(tensor=xs.tensor, offset=xs.offset,
                              ap=[xs.ap[0], [64, 64], [1, 32]])
                nc.vector.tensor_mul(out=tmp_h0[:], in0=in0, in1=in1)

            def h0_hat():
                nc.vector.tensor_mul(
                    out=bass.AP(tensor=catB.tensor, offset=catB.offset + PAD + 1,
                                ap=[catB.ap[0], [PAD, 64], [1, 32]]),
                    in0=tmp_h0[:].rearrange("p (a b) -> p a b", b=32),
                    in1=rzb_h0[0][:].rearrange("p (a b) -> p a b", b=32))

            # ---- h direction, column half 1 (the tail) -------------------
            g_chain("h", 1, {1: [h0_recip], 3: [h0_tmp], 5: [h0_hat]})
            att_h = att_t["h"][:].rearrange("p a b -> p (a b)")
            if debug:
                nc.sync.dma_start(out=taps["t_att"][:], in_=att_h[:])
            zs_h1 = z_mms_j("h", 1)
            conv_A(cps, [3])
            rzb_h1 = rz_chain_j(zs_h1)
            # apply in 4 row bands; conv row-pair rp needs image rows up to
            # 16(rp+1)+1, so emit conv rp-1 after each band.
            for ch in range(4):
                tw = work.tile([64, 512], BF16, tag="tmph1", bufs=2, name="tmph1")
                in0 = bass.AP(tensor=att_t["h"].tensor,
                              offset=att_t["h"].offset + ch * 16 * 64 + 32,
                              ap=[att_t["h"].ap[0], [64, 16], [1, 32]])
                in1 = bass.AP(tensor=xs.tensor, offset=xs.offset + ch * 16 * 64 + 32,
                              ap=[xs.ap[0], [64, 16], [1, 32]])
                nc.vector.tensor_mul(out=tw[:], in0=in0, in1=in1)
                nc.vector.tensor_mul(
                    out=bass.AP(tensor=catB.tensor,
                                offset=catB.offset + (ch * 16 + 1) * PAD + 33,
                                ap=[catB.ap[0], [PAD, 16], [1, 32]]),
                    in0=tw[:].rearrange("p (a b) -> p a b", b=32),
                    in1=rzb_h1[:, ch * 512:(ch + 1) * 512].rearrange(
                        "p (a b) -> p a b", b=32))
                if ch >= 1:
                    conv_tail(cps, [ch - 1])
            conv_tail(cps, [3])

        if debug:
            nc.sync.dma_start(out=taps["t_watt"][:], in_=hat_t["w"][:])
            nc.sync.dma_start(out=taps["t_hatt"][:],
                              in_=pad_interior_ap(catB, 0, 64))

    nc.finalize()
    return nc


def _host_prep(Wq, bq, Wk, bk, Wc, bc, Wo, bo, gamma, beta, run_mean, run_var):
    bf = ml_dtypes.bfloat16
    # Wc permuted so the contraction index is (spatial, channel)
    wcp = Wc.reshape(C, C, L).transpose(0, 2, 1).reshape(C, C * L)
    wcpt = np.ascontiguousarray(
        wcp.T.reshape(NKB, 128, 64).transpose(1, 0, 2))  # [128, 32, 64]
    inv = gamma / np.sqrt(run_var + BN_EPS)
    wo_eff = Wo * inv[:, None, None, None]
    wot = wo_eff.transpose(1, 2, 3, 0).reshape(3 * C, 9, C)  # [192, 9, 64]
    # conv image A carries [x; w_att], image B carries h_att
    return {
        "wqt": np.ascontiguousarray(np.concatenate([Wq.T, Wq.T])).astype(bf),
        "wkt": np.ascontiguousarray(np.concatenate([Wk.T, Wk.T])).astype(bf),
        "wcpt": wcpt.astype(bf),
        "woa": np.ascontiguousarray(
            np.concatenate([wot[0:64], wot[128:192]])).astype(bf),
        "wob": np.ascontiguousarray(wot[64:128]).astype(bf),
        "bqq": np.concatenate([bq, bq]).reshape(128, 1).astype(np.float32),
        "bkk": np.concatenate([bk, bk]).reshape(128, 1).astype(np.float32),
        "bc": bc.reshape(64, 1).astype(np.float32),
        "bo_eff": (bo * inv).reshape(64, 1).astype(np.float32),
        "d_vec": (beta - run_mean * inv).reshape(64, 1).astype(np.float32),
        "ident": np.concatenate([np.zeros((64, 64), np.float32),
                                 np.eye(64, dtype=np.float32)]).astype(bf),
    }


def kernel(x, Wq, bq, Wk, bk, Wc, bc, Wo, bo, gamma, beta, run_mean, run_var,
           debug=False, trace=False, trace_kwargs=None):
    x = np.asarray(x, np.float32)
    weights = _host_prep(
        np.asarray(Wq, np.float32), np.asarray(bq, np.float32),
        np.asarray(Wk, np.float32), np.asarray(bk, np.float32),
        np.asarray(Wc, np.float32), np.asarray(bc, np.float32),
        np.asarray(Wo, np.float32), np.asarray(bo, np.float32),
        np.asarray(gamma, np.float32), np.asarray(beta, np.float32),
        np.asarray(run_mean, np.float32), np.asarray(run_var, np.float32))
    key = bool(debug)
    if key not in _CACHE:
        _CACHE[key] = _build_nc(debug=debug)
    nc = _CACHE[key]
    bf = ml_dtypes.bfloat16
    in_maps = []
    for b in range(B):
        m = dict(weights)
        xr = np.ascontiguousarray(x[b].reshape(C, N)).astype(bf)
        m["xbf"] = xr
        m["x2bf"] = np.ascontiguousarray(
            np.concatenate([xr[:, 0:N // 2], xr[:, N // 2:]], axis=0))
        in_maps.append(m)
    kwargs = {}
    if trace:
        kwargs = dict(trace=True, trace_cores=[0], **(trace_kwargs or {}))
    res = run_bass_kernel_spmd(nc, in_maps, core_ids=list(range(B)), **kwargs)
    out = np.stack([res.results[b]["y"].astype(np.float32).reshape(C, L, L)
                    for b in range(B)])
    if debug or trace:
        return out, res
    return out


# revision 43
# speedup vs baseline: 1.0283x; 1.0283x over previous
"""nn_DirAttention kernel for 8 Trainium2 NeuronCores.

Strategy: data-parallel over batch (B=8, one batch element per core).
Per core, the directional attention

    ah[o,i,j] = sum_k Wc[o,k] * Qh[k,i] * Kh[k,j]   (k = C*L = 4096)

is computed by materialising G[k,(j,i)] = Kh[k,j]*Qh[k,i] per 128-row
k-block on the Vector engine (outer-product broadcast via a
column-duplicated K so every operand presents dense bf16 pairs to the
DVE -> 2x mode), then accumulating ah = Wc' @ G on the PE with even/odd
k-blocks on the two halves of the array.  Softmax over the channel
(partition) axis uses an ACT exp with per-partition bias bc, a
ones-matmul for the column sums, a 64-lane reciprocal via a DRAM
shuffle, and a DMA partition-broadcast of 1/Z.  The 3x3 conv runs as
shifted accumulating matmuls over zero-padded SBUF images, with both
image halves accumulating into one PSUM pass per output chunk.
BatchNorm is folded into the conv weights on the host.

The projections read a single x tile with parity-offset access
patterns (no materialised shifted copies), the Z sums contract K=64,
the conv's second image pass is K=64 (no zero rows), the BN shift
runs on the ACT engine, and y is written back in bf16.
"""

import sys

for _p in ("/opt/trn_rl_repo",):
    if _p not in sys.path:
        sys.path.append(_p)

import numpy as np
import ml_dtypes

import concourse.bacc as bacc
import concourse.bass as bass
import concourse.mybir as mybir
import concourse.tile as tile
from concourse.bass_utils import run_bass_kernel_spmd

BF16 = mybir.dt.bfloat16
F32 = mybir.dt.float32
B, C, L = 8, 64, 64
N = L * L  # 4096
NKB = 32  # 128-row k-blocks in the C*L contraction
BN_EPS = 1e-5
PAD = L + 2  # 66, padded row stride for the conv images

_CACHE = {}


def _build_nc(debug=False):
    nc = bacc.Bacc(target_bir_lowering=False)

    # ---- DRAM parameters -------------------------------------------------
    xbf = nc.dram_tensor("xbf", [C, N], BF16, kind="ExternalInput")
    x2bf = nc.dram_tensor("x2bf", [128, N // 2], BF16, kind="ExternalInput")
    wqk_d = nc.dram_tensor("wqk", [128, 128], BF16, kind="ExternalInput")
    wcpt = nc.dram_tensor("wcpt", [128, NKB, 64], BF16, kind="ExternalInput")
    woa = nc.dram_tensor("woa", [128, 9, 64], BF16, kind="ExternalInput")
    wob = nc.dram_tensor("wob", [64, 9, 64], BF16, kind="ExternalInput")
    bias3_d = nc.dram_tensor("bias3", [128, 3], F32, kind="ExternalInput")
    bo_d = nc.dram_tensor("bo_eff", [64, 1], F32, kind="ExternalInput")
    dv_d = nc.dram_tensor("d_vec", [64, 1], F32, kind="ExternalInput")
    ident_d = nc.dram_tensor("ident", [128, 64], BF16, kind="ExternalInput")
    y = nc.dram_tensor("y", [C, N], BF16, kind="ExternalOutput")
    taps = {}
    if debug:
        for nm, shp, dt in [
            ("t_att", [64, N], F32), ("t_z", [64, N], F32),
            ("t_hatt", [64, N], F32), ("t_watt", [64, N], F32),
        ]:
            taps[nm] = nc.dram_tensor(nm, shp, dt, kind="ExternalOutput")

    from contextlib import ExitStack
    with tile.TileContext(nc) as tc, ExitStack() as _es:
        consts = _es.enter_context(tc.tile_pool(name="consts", bufs=1))
        qk = _es.enter_context(tc.tile_pool(name="qk", bufs=1))
        work = _es.enter_context(tc.tile_pool(name="work", bufs=2))
        gpool = _es.enter_context(tc.tile_pool(name="gpool", bufs=12))
        dpool = _es.enter_context(tc.tile_pool(name="dscratch", bufs=2, space="DRAM"))

        # ---- constant loads ---------------------------------------------
        xs = consts.tile([64, N], BF16)     # x image for apply/conv stages
        x2 = consts.tile([128, N // 2], BF16)  # x col-halves stacked: fast load
        wqk_sb = consts.tile([128, 128], BF16)
        wc_sb = consts.tile([128, NKB, 64], BF16)
        woa_sb = consts.tile([128, 9, 64], BF16)
        wob_sb = consts.tile([64, 9, 64], BF16)
        bias3 = consts.tile([128, 3], F32)
        bov2 = consts.tile([128, 1], F32)
        dvv2 = consts.tile([128, 1], F32)
        ones = consts.tile([128, 1], BF16)
        ident_sb = consts.tile([128, 64], BF16)
        warm = consts.tile([128, 512], BF16)
        bqq, bkk, bcv = bias3[:, 0:1], bias3[:, 1:2], bias3[0:64, 2:3]

        # DMA rate is ~per-partition-bytes, so the proj-gating x2 image is
        # split by columns across two queues with minimal company; wc
        # arrives in k-block chunks (the attention matmuls consume them in
        # order); the [64, 4096] xs copy (h projections + apply stages,
        # needed later) trickles in behind.
        nc.sync.dma_start(out=x2[:, 0:1024], in_=x2bf[:, 0:1024])
        nc.gpsimd.dma_start(out=x2[:, 1024:2048], in_=x2bf[:, 1024:2048])
        nc.scalar.dma_start(out=wqk_sb[:], in_=wqk_d[:])
        nc.scalar.dma_start(out=bias3[:], in_=bias3_d[:])
        nc.scalar.dma_start(out=wc_sb[:, 0:4, :], in_=wcpt[:, 0:4, :])
        nc.scalar.dma_start(out=xs[:, 2048:4096], in_=xbf[:, 2048:4096])
        nc.sync.dma_start(out=wc_sb[:, 4:16, :], in_=wcpt[:, 4:16, :])
        nc.sync.dma_start(out=xs[:, 0:2048], in_=xbf[:, 0:2048])
        nc.gpsimd.dma_start(out=wc_sb[:, 16:32, :], in_=wcpt[:, 16:32, :])
        nc.gpsimd.dma_start(out=woa_sb[:], in_=woa[:])
        nc.gpsimd.dma_start(out=wob_sb[:], in_=wob[:])
        nc.gpsimd.dma_start(out=bov2[0:64], in_=bo_d[:])
        nc.gpsimd.dma_start(out=bov2[64:128], in_=bo_d[:])
        nc.gpsimd.dma_start(out=dvv2[0:64], in_=dv_d[:])
        nc.gpsimd.dma_start(out=dvv2[64:128], in_=dv_d[:])
        nc.gpsimd.dma_start(out=ident_sb[:], in_=ident_d[:])
        nc.vector.memset(ones[:], 1.0)
        nc.vector.memset(warm[:], 1.0)

        # conv image buffers (zero ring borders; interiors fully written)
        catA = consts.tile([128, PAD * PAD], BF16)  # rows 0-63 x, 64-127 w_att
        catB = consts.tile([64, PAD * PAD], BF16)   # h_att
        for t, p in ((catA, 128), (catB, 64)):
            nc.vector.memset(t[:, 0:PAD], 0.0)
            nc.vector.memset(t[:, (PAD - 1) * PAD:PAD * PAD], 0.0)
            nc.vector.memset(
                bass.AP(tensor=t.tensor, offset=t.offset + PAD,
                        ap=[t.ap[0], [PAD, L]]), 0.0)
            nc.vector.memset(
                bass.AP(tensor=t.tensor, offset=t.offset + PAD + L + 1,
                        ap=[t.ap[0], [PAD, L]]), 0.0)

        def pad_interior_ap(t, p0, p1, row0=0, nrows=L):
            base = t[p0:p1, :]
            return bass.AP(tensor=base.tensor,
                           offset=base.offset + (row0 + 1) * PAD + 1,
                           ap=[base.ap[0], [PAD, nrows], [1, L]])

        # x part of the conv image
        nc.sync.dma_start(out=pad_interior_ap(catA, 0, 64), in_=xs[:])

        # ---- projections -------------------------------------------------
        # Per direction: Q [128, 32, 64] (block kb = spatial pair, partition
        # = (parity, channel)), Kdup [128, 32, 64, 2] (K duplicated pairs).
        # The parity halves read the single x tile at +1 / +64 offsets.
        q_t = {d: qk.tile([128, NKB, 64], BF16, tag=f"q{d}", name=f"q_{d}") for d in "hw"}
        kd_t = {d: qk.tile([128, NKB, 64, 2], BF16, tag=f"k{d}", name=f"kd_{d}") for d in "hw"}

        with tc.tile_pool(name="projps", bufs=6, space="PSUM") as pps, \
             tc.tile_pool(name="warmps", bufs=1, space="PSUM") as wps:
            # clock-warming dummy matmuls (PE otherwise idles until x lands)
            wt = wps.tile([1, 512], F32, tag="warm")
            for _ in range(8):
                nc.tensor.matmul(out=wt[:], lhsT=warm[:, 0:1], rhs=warm[:],
                                 start=True, stop=True)
            # w first: its projections read the fast-loading stacked x2
            # image ((h<32, h>=32) on partition halves, clean t8-level
            # split).  h's projections read the slowly-arriving xs and run
            # during w's G phase, well off the critical path.
            for d in "wh":
                for t8 in ((0, 2, 1, 3) if d == "w" else range(4)):
                    for proj in "qk":
                        wcol = 0 if proj == "q" else 64
                        bias = bqq if proj == "q" else bkk
                        ps = pps.tile([128, 8, 64], F32, tag="proj")
                        for half in range(2):  # 4 g per matmul
                            g0 = t8 * 8 + half * 4
                            for par in range(2):
                                if d == "h":
                                    rhs = bass.AP(
                                        tensor=xs.tensor,
                                        offset=xs.offset + 2 * g0 + par,
                                        ap=[xs.ap[0], [2, 4], [64, 64]])
                                    nc.tensor.matmul(
                                        out=ps[par * 64:(par + 1) * 64,
                                               half * 4:(half + 1) * 4, :],
                                        lhsT=wqk_sb[0:64, wcol:wcol + 64],
                                        rhs=rhs,
                                        start=True, stop=True,
                                        skip_group_check=True,
                                        tile_position=(0, par * 64))
                                    continue
                                hi = t8 >= 2
                                p0 = 64 * hi
                                xb = x2[p0:p0 + 64, :]
                                rhs = bass.AP(
                                    tensor=xb.tensor,
                                    offset=xb.offset + 128 * g0 + 64 * par
                                    - hi * 2048,
                                    ap=[xb.ap[0], [128, 4], [1, 64]])
                                nc.tensor.matmul(
                                    out=ps[par * 64:(par + 1) * 64,
                                           half * 4:(half + 1) * 4, :],
                                    lhsT=wqk_sb[p0:p0 + 64, wcol:wcol + 64],
                                    rhs=rhs,
                                    start=True, stop=True,
                                    skip_group_check=True,
                                    tile_position=(p0, par * 64))
                        if proj == "q":
                            nc.scalar.activation(
                                out=q_t[d][:, t8 * 8:(t8 + 1) * 8, :], in_=ps[:],
                                func=mybir.ActivationFunctionType.Identity,
                                bias=bias[:], scale=1.0)
                        else:
                            for dup in range(2):
                                dst = bass.AP(
                                    tensor=kd_t[d].tensor,
                                    offset=kd_t[d].offset + t8 * 8 * 128 + dup,
                                    ap=[kd_t[d].ap[0], [128, 8], [2, 64]])
                                nc.scalar.activation(
                                    out=dst, in_=ps[:],
                                    func=mybir.ActivationFunctionType.Identity,
                                    bias=bias[:], scale=1.0)

        # ---- attention + softmax + apply + conv --------------------------
        att_t = {d: work.tile([64, L, L], BF16, tag=f"att{d}", bufs=1,
                              name=f"att_{d}") for d in "hw"}
        hat_t = {"w": work.tile([64, N], BF16, tag="hatw", bufs=1, name="hat_w")}

        cv_tiles = {}

        def conv_A(cps, rps):
            # catA half (x + h_att): runs during the w Z-chain latency
            for rp in rps:
                cv_tiles[rp] = cv = cps.tile([128, 512], F32, tag="cv", name="cv")
                for tap in range(9):
                    dy, dx = tap // 3, tap % 3
                    for half in range(2):
                        r = rp * 2 + half
                        off = (r * 8 + dy) * PAD + dx
                        rhs = bass.AP(tensor=catA.tensor, offset=catA.offset + off,
                                      ap=[catA.ap[0], [PAD, 8], [1, 64]])
                        nc.tensor.matmul(out=cv[half * 64:(half + 1) * 64, :],
                                         lhsT=woa_sb[:, tap, :], rhs=rhs,
                                         start=(tap == 0), stop=False,
                                         skip_group_check=True,
                                         tile_position=(0, half * 64))

        def conv_B(cps, rps, right):
            # catB half (h_att, K=64) accumulates onto the A-pass result.
            # The left column group (out cols 0-30) only needs the jh0 half
            # of h_att and runs during the final G chain; the right group
            # (31-63) is tail-gated.
            c0, cn = (31, 33) if right else (0, 31)
            for rp in rps:
                cv = cv_tiles[rp]
                for tap in range(9):
                    dy, dx = tap // 3, tap % 3
                    for half in range(2):
                        r = rp * 2 + half
                        off = (r * 8 + dy) * PAD + dx + c0
                        rhs = bass.AP(tensor=catB.tensor, offset=catB.offset + off,
                                      ap=[catB.ap[0], [PAD, 8], [1, cn]])
                        base = cv[half * 64:(half + 1) * 64, :]
                        out = bass.AP(tensor=base.tensor, offset=base.offset + c0,
                                      ap=[base.ap[0], [64, 8], [1, cn]])
                        nc.tensor.matmul(out=out,
                                         lhsT=wob_sb[:, tap, :], rhs=rhs,
                                         start=False,
                                         stop=(right and tap == 8),
                                         skip_group_check=True,
                                         tile_position=(0, half * 64))
                if not right:
                    continue
                ysb = work.tile([128, 512], F32, tag="ysb", name="ysb")
                nc.scalar.activation(out=ysb[0:64, :], in_=cv[0:64, :],
                                     func=mybir.ActivationFunctionType.Relu,
                                     bias=bov2[0:64], scale=1.0)
                nc.scalar.activation(out=ysb[64:128, :], in_=cv[64:128, :],
                                     func=mybir.ActivationFunctionType.Relu,
                                     bias=bov2[64:128], scale=1.0)
                ysb2 = work.tile([128, 512], BF16, tag="ysb2", name="ysb2")
                nc.scalar.activation(out=ysb2[:], in_=ysb[:],
                                     func=mybir.ActivationFunctionType.Identity,
                                     bias=dvv2[:], scale=1.0)
                nc.sync.dma_start(out=y[:, (2 * rp) * 512:(2 * rp + 1) * 512],
                                  in_=ysb2[0:64, :])
                nc.sync.dma_start(out=y[:, (2 * rp + 1) * 512:(2 * rp + 2) * 512],
                                  in_=ysb2[64:128, :])

        with tc.tile_pool(name="ahps", bufs=1, space="PSUM") as aps, \
             tc.tile_pool(name="cvps", bufs=4, space="PSUM") as cps:

            def g_chain(d, jh, interleave=()):
                # G production + ah accumulation for one (direction, column
                # half).  `interleave` maps kbp -> [fn] emitting deferred DVE
                # ops (reciprocals / apply muls of the PREVIOUS chain) into
                # the middle of this chain's G stream, so their input DMAs
                # have landed by the time the in-order DVE queue reaches
                # them.
                last = (d == "h" and jh == 1)
                q, kd = q_t[d], kd_t[d]
                ah = aps.tile([128, 2048], F32, tag="ah", name="ah")
                inter = dict(interleave)
                for kbp in range(NKB // 2):
                    for fn in inter.get(kbp, ()):
                        fn()
                    grhs = {}
                    for half in range(2):
                        kb = kbp * 2 + half
                        g = gpool.tile([128, 32, 64], BF16, tag="g", name=f"g{half}")
                        # G[k, j, i] = K[k,j] * Q[k,i] (2x-mode paired APs)
                        in0 = bass.AP(
                            tensor=kd.tensor,
                            offset=kd.offset + kb * 128 + jh * 64,
                            ap=[kd.ap[0], [2, 32], [0, 32], [1, 2]])
                        in1 = bass.AP(
                            tensor=q.tensor, offset=q.offset + kb * 64,
                            ap=[q.ap[0], [0, 32], [2, 32], [1, 2]])
                        gout = bass.AP(
                            tensor=g.tensor, offset=g.offset,
                            ap=[g.ap[0], [64, 32], [2, 32], [1, 2]])
                        nc.vector.tensor_mul(out=gout, in0=in0, in1=in1)
                        grhs[half] = g[:].rearrange("p a b -> p (a b)")
                    for ns in range(4):
                        for half in range(2):
                            kb = kbp * 2 + half
                            opart = 0 if last else half * 64
                            nc.tensor.matmul(
                                out=ah[opart:opart + 64, ns * 512:(ns + 1) * 512],
                                lhsT=wc_sb[:, kb, :],
                                rhs=grhs[half][:, ns * 512:(ns + 1) * 512],
                                start=(kbp == 0 and (half == 0 or not last)),
                                stop=(kbp == NKB // 2 - 1 and ns == 3),
                                skip_group_check=True,
                                tile_position=(0, opart))
                if not last:
                    # fold the odd-half partial into the even-half region
                    # via an identity matmul (ACT copy, same partitions).
                    fold = work.tile([128, 2048], BF16, tag="fold", name="fold", bufs=2)
                    nc.scalar.copy(out=fold[64:128, :], in_=ah[64:128, :])
                    for ns in range(4):
                        nc.tensor.matmul(
                            out=ah[0:64, ns * 512:(ns + 1) * 512],
                            lhsT=ident_sb[64:128, :],
                            rhs=fold[64:128, ns * 512:(ns + 1) * 512],
                            start=False, stop=True,
                            skip_group_check=True,
                            tile_position=(64, 0))
                # exp with transposed read: ah[(j,i)] -> att[(i, j)]
                src = bass.AP(tensor=ah.tensor, offset=ah.offset,
                              ap=[[ah.ap[0][0], 64], [1, 64], [64, 32]])
                nc.scalar.activation(
                    out=att_t[d][:, :, jh * 32:(jh + 1) * 32], in_=src,
                    func=mybir.ActivationFunctionType.Exp,
                    bias=bcv[:], scale=1.0)

            def z_sums(att3, chunks, zs_dst):
                # Z column sums: K=64 ones-matmuls into four disjoint
                # column-groups (psum rows 0/32/64/96) of ONE bank-wide
                # tile, then a single strided DMA spreads them into the
                # [rows, 32/64-wide] zs block for the reciprocal.
                zt4 = cps.tile([128, 512], F32, tag="cv", name="zt4")
                for c4, (off, apf) in enumerate(chunks):
                    rhs = bass.AP(tensor=att3.tensor, offset=att3.offset + off,
                                  ap=[att3.ap[0]] + apf)
                    nc.tensor.matmul(out=zt4[32 * c4:32 * c4 + 1, :],
                                     lhsT=ones[0:64], rhs=rhs,
                                     start=True, stop=True,
                                     skip_group_check=True,
                                     tile_position=(0, 32 * c4))
                zsp = work.tile([128, 512], F32, tag="zsp", bufs=2, name="zsp")
                nc.scalar.copy(out=zsp[:], in_=zt4[:])
                nc.scalar.dma_start(
                    out=zs_dst,
                    in_=bass.AP(tensor=zsp.tensor, offset=zsp.offset,
                                ap=[[zsp.ap[0][0] * 32, 4], [1, 512]]))

            def z_mms_j(d, jh):
                # per-column-half Z sums, (i-major, 32 j) layout
                zs = work.tile([64, 32], F32, tag="zsj", bufs=2, name="zsj")
                z_sums(att_t[d],
                       [(c4 * 16 * 64 + jh * 32, [[64, 16], [1, 32]])
                        for c4 in range(4)], zs[:])
                return zs

            def rz_chain_j(zs):
                # reciprocal + DRAM-broadcast of 1/Z for one column half
                rzs = work.tile([64, 32], BF16, tag="rzsj", bufs=2, name="rzsj")
                with nc.allow_low_precision(reason="1/Z multiplier in bf16"):
                    nc.vector.reciprocal(out=rzs[:], in_=zs[:])
                rz = dpool.tile([64, 32], BF16, tag="rzdj")
                nc.scalar.dma_start(out=rz[:], in_=rzs[:])
                rzb = work.tile([64, 2048], BF16, tag="rzbj", bufs=2, name="rzbj")
                for qi, queue in enumerate((nc.sync, nc.scalar)):
                    queue.dma_start(
                        out=rzb[:, qi * 1024:(qi + 1) * 1024],
                        in_=bass.AP(tensor=rz.tensor, offset=rz.offset + qi * 1024,
                                    ap=[[0, 64], [32, 32], [1, 32]]))
                return rzb

            # ---- w direction: full-width softmax chain, deferred into the
            # h/jh0 G stream ----------------------------------------------
            g_chain("w", 0)
            g_chain("w", 1)
            att_w = att_t["w"][:].rearrange("p a b -> p (a b)")
            if debug:
                nc.sync.dma_start(out=taps["t_z"][:], in_=att_w[:])
            zs_w = work.tile([64, 64], F32, tag="zsw", bufs=1)
            for hb in range(2):
                z_sums(att_t["w"],
                       [((hb * 4 + c4) * 512, [[1, 512]]) for c4 in range(4)],
                       zs_w[hb * 32:(hb + 1) * 32, :])
            rzb_w = work.tile([64, N], BF16, tag="rzbw", bufs=1)
            tmp_w = work.tile([64, N], BF16, tag="tmpw", bufs=1)

            def w_recip():
                rzs = work.tile([64, 64], BF16, tag="rzsw", bufs=1)
                with nc.allow_low_precision(reason="1/Z multiplier in bf16"):
                    nc.vector.reciprocal(out=rzs[:], in_=zs_w[:])
                rz = dpool.tile([64, 64], BF16, tag="rzdw")
                nc.scalar.dma_start(out=rz[:], in_=rzs[:])
                for ch in range(2):
                    sl = slice(ch * 2048, (ch + 1) * 2048)
                    nc.sync.dma_start(
                        out=rzb_w[:, sl],
                        in_=bass.AP(tensor=rz.tensor, offset=rz.offset + ch * 2048,
                                    ap=[[0, 64], [64, 32], [1, 64]]))

            def w_tmp():
                nc.vector.tensor_mul(out=tmp_w[:], in0=att_w[:], in1=xs[:])

            def w_hat(ch):
                sl = slice(ch * 2048, (ch + 1) * 2048)
                nc.vector.tensor_mul(out=hat_t["w"][:, sl], in0=tmp_w[:, sl],
                                     in1=rzb_w[:, sl])
                nc.sync.dma_start(
                    out=pad_interior_ap(catA, 64, 128, row0=ch * 32, nrows=32),
                    in_=hat_t["w"][:, sl])

            # ---- h direction, column half 0 ------------------------------
            g_chain("h", 0, {1: [w_recip], 2: [w_tmp],
                             4: [lambda: w_hat(0)], 5: [lambda: w_hat(1)]})
            zs_h0 = z_mms_j("h", 0)
            conv_A(cps, [0, 1, 2])
            conv_A3 = lambda: conv_A(cps, [3])  # noqa: E731
            rzb_h0 = [None]
            tmp_h0 = work.tile([64, 2048], BF16, tag="tmph0", bufs=1)

            def h0_recip():
                rzb_h0[0] = rz_chain_j(zs_h0)

            def h0_tmp():
                in0 = bass.AP(tensor=att_t["h"].tensor, offset=att_t["h"].offset,
                              ap=[att_t["h"].ap[0], [64, 64], [1, 32]])
                in1 = bass.AP(tensor=xs.tensor, offset=xs.offset,
                              ap=[xs.ap[0], [64, 64], [1, 32]])
                nc.vector.tensor_mul(out=tmp_h0[:], in0=in0, in1=in1)

            def h0_hat():
                nc.vector.tensor_mul(
                    out=bass.AP(tensor=catB.tensor, offset=catB.offset + PAD + 1,
                                ap=[catB.ap[0], [PAD, 64], [1, 32]]),
                    in0=tmp_h0[:].rearrange("p (a b) -> p a b", b=32),
                    in1=rzb_h0[0][:].rearrange("p (a b) -> p a b", b=32))

            # ---- h direction, column half 1 (the tail) -------------------
            # the left conv-B column groups only need jh0's h_att columns
            # and run inside this chain's G stream
            g_chain("h", 1, {1: [h0_recip], 3: [h0_tmp], 5: [h0_hat],
                             8: [lambda: conv_B(cps, [0, 1], False)],
                             11: [lambda: conv_B(cps, [2], False)]})
            att_h = att_t["h"][:].rearrange("p a b -> p (a b)")
            if debug:
                nc.sync.dma_start(out=taps["t_att"][:], in_=att_h[:])
            zs_h1 = z_mms_j("h", 1)
            conv_A3()
            conv_B(cps, [3], False)
            rzb_h1 = rz_chain_j(zs_h1)
            # apply in 4 row bands (att*x products first -- they only need
            # the exp -- then the 1/Z muls as the broadcast lands); conv
            # row-pair rp needs image rows up to 16(rp+1)+1, so emit conv
            # rp-1 after each band.
            tws = []
            for ch in range(4):
                tw = work.tile([64, 512], BF16, tag="tmph1", bufs=4, name="tmph1")
                in0 = bass.AP(tensor=att_t["h"].tensor,
                              offset=att_t["h"].offset + ch * 16 * 64 + 32,
                              ap=[att_t["h"].ap[0], [64, 16], [1, 32]])
                in1 = bass.AP(tensor=xs.tensor, offset=xs.offset + ch * 16 * 64 + 32,
                              ap=[xs.ap[0], [64, 16], [1, 32]])
                nc.vector.tensor_mul(out=tw[:], in0=in0, in1=in1)
                tws.append(tw)
            for ch in range(4):
                nc.vector.tensor_mul(
                    out=bass.AP(tensor=catB.tensor,
                                offset=catB.offset + (ch * 16 + 1) * PAD + 33,
                                ap=[catB.ap[0], [PAD, 16], [1, 32]]),
                    in0=tws[ch][:].rearrange("p (a b) -> p a b", b=32),
                    in1=rzb_h1[:, ch * 512:(ch + 1) * 512].rearrange(
                        "p (a b) -> p a b", b=32))
                if ch >= 1:
                    conv_B(cps, [ch - 1], True)
            conv_B(cps, [3], True)

        if debug:
            nc.sync.dma_start(out=taps["t_watt"][:], in_=hat_t["w"][:])
            nc.sync.dma_start(out=taps["t_hatt"][:],
                              in_=pad_interior_ap(catB, 0, 64))

    nc.finalize()
    return nc


def _host_prep(Wq, bq, Wk, bk, Wc, bc, Wo, bo, gamma, beta, run_mean, run_var):
    bf = ml_dtypes.bfloat16
    # Wc permuted so the contraction index is (spatial, channel)
    wcp = Wc.reshape(C, C, L).transpose(0, 2, 1).reshape(C, C * L)
    wcpt = np.ascontiguousarray(
        wcp.T.reshape(NKB, 128, 64).transpose(1, 0, 2))  # [128, 32, 64]
    inv = gamma / np.sqrt(run_var + BN_EPS)
    wo_eff = Wo * inv[:, None, None, None]
    wot = wo_eff.transpose(1, 2, 3, 0).reshape(3 * C, 9, C)  # [192, 9, 64]
    # conv image A carries [x; w_att], image B carries h_att
    wq2 = np.concatenate([Wq.T, Wq.T])  # [128, 64]
    wk2 = np.concatenate([Wk.T, Wk.T])
    bias3 = np.stack([np.concatenate([bq, bq]), np.concatenate([bk, bk]),
                      np.concatenate([bc, bc])], axis=1)  # [128, 3]
    return {
        "wqk": np.ascontiguousarray(
            np.concatenate([wq2, wk2], axis=1)).astype(bf),
        "wcpt": wcpt.astype(bf),
        "woa": np.ascontiguousarray(
            np.concatenate([wot[0:64], wot[128:192]])).astype(bf),
        "wob": np.ascontiguousarray(wot[64:128]).astype(bf),
        "bias3": np.ascontiguousarray(bias3).astype(np.float32),
        "bo_eff": (bo * inv).reshape(64, 1).astype(np.float32),
        "d_vec": (beta - run_mean * inv).reshape(64, 1).astype(np.float32),
        "ident": np.concatenate([np.zeros((64, 64), np.float32),
                                 np.eye(64, dtype=np.float32)]).astype(bf),
    }


def kernel(x, Wq, bq, Wk, bk, Wc, bc, Wo, bo, gamma, beta, run_mean, run_var,
           debug=False, trace=False, trace_kwargs=None):
    x = np.asarray(x, np.float32)
    weights = _host_prep(
        np.asarray(Wq, np.float32), np.asarray(bq, np.float32),
        np.asarray(Wk, np.float32), np.asarray(bk, np.float32),
        np.asarray(Wc, np.float32), np.asarray(bc, np.float32),
        np.asarray(Wo, np.float32), np.asarray(bo, np.float32),
        np.asarray(gamma, np.float32), np.asarray(beta, np.float32),
        np.asarray(run_mean, np.float32), np.asarray(run_var, np.float32))
    key = bool(debug)
    if key not in _CACHE:
        _CACHE[key] = _build_nc(debug=debug)
    nc = _CACHE[key]
    bf = ml_dtypes.bfloat16
    in_maps = []
    for b in range(B):
        m = dict(weights)
        xr = np.ascontiguousarray(x[b].reshape(C, N)).astype(bf)
        m["xbf"] = xr
        m["x2bf"] = np.ascontiguousarray(
            np.concatenate([xr[:, 0:N // 2], xr[:, N // 2:]], axis=0))
        in_maps.append(m)
    kwargs = {}
    if trace:
        kwargs = dict(trace=True, trace_cores=[0], **(trace_kwargs or {}))
    res = run_bass_kernel_spmd(nc, in_maps, core_ids=list(range(B)), **kwargs)
    out = np.stack([res.results[b]["y"].astype(np.float32).reshape(C, L, L)
                    for b in range(B)])
    if debug or trace:
        return out, res
    return out
